# revision 2
# baseline (speedup 1.0000x reference)
"""Trainium2 Bass kernel for nn_CNEncoder_32658931319277.

Pipeline (see reference): per-spot stats over two [20000, 3000] f32
matrices, a tiny MCMC + graph smoothing on the [20000] spot dimension,
then an affine per-row rescale of `reconstructed_features` into the
output. The heavy O(N*G) work runs on 8 NeuronCores, data-parallel over
rows (2500 rows/core):

  pass 1 (device): per-row sum/min/max/sum-of-squares of rf + row sum
      of norm_x  -> tiny [128, 128] stats output per core.
  host glue (O(N+E), numpy/jax-cpu): MCMC with the exact jax threefry
      draws (data-independent), scatter-mean over edges, final_states,
      global min/max/mean algebra -> per-row A[i], global B with
      out[i,j] = rf[i,j] * A[i] + B.
  pass 2 (device): the affine map, one fused tensor_scalar per tile.
"""

import numpy as np

N = 20000
G = 3000
NCORES = 8
RPC = N // NCORES  # 2500 rows per core
P = 128
NT = (RPC + P - 1) // P  # 20 tiles, last tile 68 rows
EPS = 1e-8

_STATE_PRIOR = np.array([0.1, 0.8, 0.1], dtype=np.float32)
_TRANS = np.array(
    [[0.98, 0.01, 0.01], [0.01, 0.98, 0.01], [0.01, 0.01, 0.98]],
    dtype=np.float32,
)

_cache = {}


# --------------------------------------------------------------------------
# BIR post-pass: this walrus build accepts only ONE sync-wait command per
# instruction ("Too many sync wait commands" otherwise). Hoist excess waits
# into standalone single-wait EventSemaphore instructions on the same engine.
# --------------------------------------------------------------------------
def _split_waits(nc, max_waits=1):
    import concourse.mybir as mybir

    n = 0
    for f in nc.m.functions:
        for bb in f.blocks:
            new_insts = []
            changed = False
            for inst in bb.instructions:
                si = getattr(inst, "sync_info", None)
                waits = list(si.on_wait) if si is not None and si.on_wait else []
                if len(waits) > max_waits:
                    hoist, keep = waits[:-max_waits], waits[-max_waits:]
                    for w in hoist:
                        n += 1
                        new_insts.append(
                            mybir.InstEventSemaphore(
                                name=f"wsplit-{n}-{inst.name}",
                                engine=inst.engine,
                                ins=[],
                                outs=[],
                                sync_info=mybir.SyncInfo(on_wait=[w], on_update=[]),
                            )
                        )
                    si.on_wait = keep
                    changed = True
                new_insts.append(inst)
            if changed:
                bb.instructions[:] = new_insts
    return n


def _build_pass1():
    import concourse.bass as bass
    import concourse.tile as tile
    from concourse import mybir

    DT = mybir.dt.float32
    X = mybir.AxisListType.X
    nc = bass.Bass("TRN2", target_bir_lowering=False, debug=False, num_devices=NCORES)
    rf = nc.dram_tensor("rf", [RPC, G], DT, kind="ExternalInput")
    nx = nc.dram_tensor("nx", [RPC, G], DT, kind="ExternalInput")
    stats = nc.dram_tensor("stats", [P, 128], DT, kind="ExternalOutput")
    # stats columns: t in [0,20): rfsum=t, nxsum=20+t, rfmin=40+t,
    # rfmax=60+t, rf sumsq=80+t  (row p of tile t <-> input row t*128+p)
    with tile.TileContext(nc) as tc:
        with (
            tc.tile_pool(name="io", bufs=3) as pool,
            tc.tile_pool(name="scratch", bufs=2) as qpool,
            tc.tile_pool(name="parts", bufs=4) as ppool,
            tc.tile_pool(name="stat", bufs=1) as spool,
        ):
            st = spool.tile([P, 128], DT)
            nc.vector.memset(st[:], 0.0)
            for t in range(NT):
                cur = min(P, RPC - t * P)
                trf = pool.tile([P, G], DT)
                nc.sync.dma_start(out=trf[:cur], in_=rf[t * P : t * P + cur])
                tnx = pool.tile([P, G], DT)
                nc.sync.dma_start(out=tnx[:cur], in_=nx[t * P : t * P + cur])
                part = ppool.tile([P, 16], DT)
                # two-level row sums (better f32 accuracy than one pass)
                nc.vector.reduce_sum(
                    out=part[:cur, 0:8],
                    in_=trf[:cur, :].rearrange("p (a b) -> p a b", b=G // 8),
                    axis=X,
                )
                nc.vector.reduce_sum(out=st[:cur, t : t + 1], in_=part[:cur, 0:8], axis=X)
                nc.vector.reduce_sum(
                    out=part[:cur, 8:16],
                    in_=tnx[:cur, :].rearrange("p (a b) -> p a b", b=G // 8),
                    axis=X,
                )
                nc.vector.reduce_sum(
                    out=st[:cur, 20 + t : 21 + t], in_=part[:cur, 8:16], axis=X
                )
                nc.vector.tensor_reduce(
                    out=st[:cur, 40 + t : 41 + t],
                    in_=trf[:cur, :],
                    op=mybir.AluOpType.min,
                    axis=X,
                )
                nc.vector.tensor_reduce(
                    out=st[:cur, 60 + t : 61 + t],
                    in_=trf[:cur, :],
                    op=mybir.AluOpType.max,
                    axis=X,
                )
                sq = qpool.tile([P, G], DT)
                nc.scalar.activation(
                    out=sq[:cur],
                    in_=trf[:cur],
                    func=mybir.ActivationFunctionType.Square,
                    accum_out=st[:cur, 80 + t : 81 + t],
                )
            nc.sync.dma_start(out=stats[:], in_=st[:])
    _split_waits(nc)
    return nc


def _build_pass2():
    import concourse.bass as bass
    import concourse.tile as tile
    from concourse import mybir

    DT = mybir.dt.float32
    nc = bass.Bass("TRN2", target_bir_lowering=False, debug=False, num_devices=NCORES)
    rf = nc.dram_tensor("rf", [RPC, G], DT, kind="ExternalInput")
    coef = nc.dram_tensor("coef", [RPC, 2], DT, kind="ExternalInput")
    out = nc.dram_tensor("out", [RPC, G], DT, kind="ExternalOutput")
    with tile.TileContext(nc) as tc:
        with tc.tile_pool(name="io", bufs=3) as pool:
            for t in range(NT):
                cur = min(P, RPC - t * P)
                trf = pool.tile([P, G], DT)
                nc.sync.dma_start(out=trf[:cur], in_=rf[t * P : t * P + cur])
                tcf = pool.tile([P, 2], DT)
                nc.sync.dma_start(out=tcf[:cur], in_=coef[t * P : t * P + cur])
                to = pool.tile([P, G], DT)
                nc.vector.tensor_scalar(
                    out=to[:cur],
                    in0=trf[:cur],
                    scalar1=tcf[:cur, 0:1],
                    scalar2=tcf[:cur, 1:2],
                    op0=mybir.AluOpType.mult,
                    op1=mybir.AluOpType.add,
                )
                nc.sync.dma_start(out=out[t * P : t * P + cur], in_=to[:cur])
    _split_waits(nc)
    return nc


def _get_nc(which):
    if which not in _cache:
        _cache[which] = _build_pass1() if which == 1 else _build_pass2()
    return _cache[which]


def _random_draws():
    """The reference's MCMC random draws (jax threefry, key 42) are
    data-independent -> generate them once, bit-exactly, on the CPU."""
    if "draws" in _cache:
        return _cache["draws"]
    import jax

    cpu = jax.devices("cpu")[0]
    with jax.default_device(cpu):
        key = jax.random.key(42)
        k0, kloop = jax.random.split(key)
        states0 = np.asarray(jax.random.randint(k0, (N,), 0, 3))
        keys = jax.random.split(kloop, 20)
        cands = np.empty((20, N), np.int32)
        us = np.empty((20, N), np.float32)
        for t in range(20):
            k1, k2 = jax.random.split(keys[t])
            cands[t] = np.asarray(jax.random.randint(k1, (N,), 0, 3))
            us[t] = np.asarray(jax.random.uniform(k2, (N,)))
    _cache["draws"] = (states0, cands, us)
    return _cache["draws"]


def _mcmc_final_states(rfsum64, edge_index):
    """Replicates reference._mcmc_state_probs + scatter_mean + argmax.

    Output feeds the result only through a ~1e-11 relative term (the EPS in
    copy_sum + EPS), so f32 rounding differences here are immaterial; the
    random bits ARE exact."""
    m64 = rfsum64 / G
    mu = m64.mean()
    sd = m64.std()
    spot_mean = ((m64 - mu) / (sd + EPS)).astype(np.float32)
    m3 = m64[:3].astype(np.float32)

    d = spot_mean[None, :] - m3[:, None]
    L = np.exp(np.float32(-0.5) * d * d).sum(axis=1, dtype=np.float32)

    states0, cands, us = _random_draws()
    states = states0.copy()
    counts = np.zeros((N, 3), np.float32)
    idx = np.arange(N)
    for t in range(20):
        cand = cands[t]
        acc = (
            (_STATE_PRIOR[cand] / _STATE_PRIOR[states])
            * (L[cand] / L[states])
            * (_TRANS[states, cand] / _TRANS[cand, states])
        )
        states = np.where(us[t] < acc, cand, states)
        counts[idx, states] += 1.0

    row, col = np.asarray(edge_index[0]), np.asarray(edge_index[1])
    sums = np.zeros((N, 3), np.float32)
    np.add.at(sums, col, counts[row])
    cnt = np.bincount(col, minlength=N).astype(np.float32)
    smoothed = sums / np.maximum(cnt, 1.0)[:, None]
    return (np.argmax(smoothed, axis=1) + 1.0).astype(np.float32)


def kernel(norm_x, reconstructed_features, edge_index):
    from concourse.bass_utils import run_bass_kernel_spmd

    rf = np.ascontiguousarray(np.asarray(reconstructed_features, dtype=np.float32))
    nx = np.ascontiguousarray(np.asarray(norm_x, dtype=np.float32))
    core_ids = list(range(NCORES))

    in1 = [
        {"rf": rf[c * RPC : (c + 1) * RPC], "nx": nx[c * RPC : (c + 1) * RPC]}
        for c in range(NCORES)
    ]
    r1 = run_bass_kernel_spmd(_get_nc(1), in1, core_ids)

    def unpack(col0):
        full = np.concatenate(
            [
                r1.results[c]["stats"][:, col0 : col0 + 20].T.reshape(-1)[:RPC]
                for c in range(NCORES)
            ]
        )
        return full.astype(np.float64)

    rfsum = unpack(0)
    nxsum = unpack(20)
    rfmin = unpack(40)
    rfmax = unpack(60)
    sumsq = unpack(80)

    fs = _mcmc_final_states(rfsum, edge_index).astype(np.float64)

    # out[i,j] = rf[i,j]*scale_i normalized:  (x - mn)/(mx - mn + EPS) *
    # (1.2 mx - 0.8 mn) + 0.8 mn, all divided by its global mean
    scale = fs * nxsum / (fs * rfsum + EPS)
    mn = np.minimum(scale * rfmin, scale * rfmax).min()
    mx = np.maximum(scale * rfmin, scale * rfmax).max()
    a = (1.2 * mx - 0.8 * mn) / (mx - mn + EPS)
    b = 0.8 * mn - mn * a
    mean_nc0 = (scale * rfsum).sum() / (float(N) * float(G))
    m_all = a * mean_nc0 + b
    A = (scale * a / m_all).astype(np.float32)
    B = np.float32(b / m_all)

    coef = np.empty((N, 2), np.float32)
    coef[:, 0] = A
    coef[:, 1] = B

    in2 = [
        {"rf": rf[c * RPC : (c + 1) * RPC], "coef": coef[c * RPC : (c + 1) * RPC]}
        for c in range(NCORES)
    ]
    r2 = run_bass_kernel_spmd(_get_nc(2), in2, core_ids)
    out = np.concatenate([r2.results[c]["out"] for c in range(NCORES)], axis=0)

    reg_loss = np.float32(sumsq.sum() * 1e-4)
    return out, reg_loss


# revision 4
# speedup vs baseline: 1.1629x; 1.1629x over previous
"""Trainium2 Bass kernel for nn_CNEncoder_32658931319277.

Pipeline (see reference): per-spot stats over two [20000, 3000] f32
matrices, a tiny MCMC + graph smoothing on the [20000] spot dimension,
then an affine per-row rescale of `reconstructed_features` into the
output. The heavy O(N*G) work runs on 8 NeuronCores, data-parallel over
rows (2500 rows/core):

  pass 1 (device): per-row sum/min/max/sum-of-squares of rf + row sum
      of norm_x  -> tiny [128, 128] stats output per core.
  host glue (O(N+E), numpy/jax-cpu): MCMC with the exact jax threefry
      draws (data-independent), scatter-mean over edges, final_states,
      global min/max/mean algebra -> per-row A[i], global B with
      out[i,j] = rf[i,j] * A[i] + B.
  pass 2 (device): the affine map, one fused tensor_scalar per tile.
"""

import numpy as np

N = 20000
G = 3000
NCORES = 8
RPC = N // NCORES  # 2500 rows per core
P = 128
NT = (RPC + P - 1) // P  # 20 tiles, last tile 68 rows
EPS = 1e-8

_STATE_PRIOR = np.array([0.1, 0.8, 0.1], dtype=np.float32)
_TRANS = np.array(
    [[0.98, 0.01, 0.01], [0.01, 0.98, 0.01], [0.01, 0.01, 0.98]],
    dtype=np.float32,
)

_cache = {}


# --------------------------------------------------------------------------
# BIR post-pass: this walrus build accepts only ONE sync-wait command per
# instruction ("Too many sync wait commands" otherwise). Hoist excess waits
# into standalone single-wait EventSemaphore instructions on the same engine.
# --------------------------------------------------------------------------
def _split_waits(nc, max_waits=1):
    import concourse.mybir as mybir

    n = 0
    for f in nc.m.functions:
        for bb in f.blocks:
            new_insts = []
            changed = False
            for inst in bb.instructions:
                si = getattr(inst, "sync_info", None)
                waits = list(si.on_wait) if si is not None and si.on_wait else []
                if len(waits) > max_waits:
                    hoist, keep = waits[:-max_waits], waits[-max_waits:]
                    for w in hoist:
                        n += 1
                        new_insts.append(
                            mybir.InstEventSemaphore(
                                name=f"wsplit-{n}-{inst.name}",
                                engine=inst.engine,
                                ins=[],
                                outs=[],
                                sync_info=mybir.SyncInfo(on_wait=[w], on_update=[]),
                            )
                        )
                    si.on_wait = keep
                    changed = True
                new_insts.append(inst)
            if changed:
                bb.instructions[:] = new_insts
    return n


def _build_pass1():
    import concourse.bass as bass
    import concourse.tile as tile
    from concourse import mybir

    DT = mybir.dt.float32
    X = mybir.AxisListType.X
    nc = bass.Bass("TRN2", target_bir_lowering=False, debug=False, num_devices=NCORES)
    rf = nc.dram_tensor("rf", [RPC, G], DT, kind="ExternalInput")
    nx = nc.dram_tensor("nx", [RPC, G], DT, kind="ExternalInput")
    stats = nc.dram_tensor("stats", [P, 128], DT, kind="ExternalOutput")
    # stats columns: t in [0,20): rfsum=t, nxsum=20+t, rfmin=40+t,
    # rfmax=60+t, rf sumsq=80+t  (row p of tile t <-> input row t*128+p)
    with tile.TileContext(nc) as tc:
        with (
            tc.tile_pool(name="io", bufs=3) as pool,
            tc.tile_pool(name="scratch", bufs=2) as qpool,
            tc.tile_pool(name="stat", bufs=1) as spool,
        ):
            st = spool.tile([P, 128], DT)
            nc.vector.memset(st[:], 0.0)
            for t in range(NT):
                cur = min(P, RPC - t * P)
                trf = pool.tile([P, G], DT)
                nc.sync.dma_start(out=trf[:cur], in_=rf[t * P : t * P + cur])
                tnx = pool.tile([P, G], DT)
                nc.sync.dma_start(out=tnx[:cur], in_=nx[t * P : t * P + cur])
                # row sums + sum of squares on ACT (Copy/Square with
                # fp32 accumulate); row min/max on DVE. DMA-bound overall.
                sc = qpool.tile([P, G], DT, tag="sq")
                nc.scalar.activation(
                    out=sc[:cur],
                    in_=trf[:cur],
                    func=mybir.ActivationFunctionType.Copy,
                    accum_out=st[:cur, t : t + 1],
                )
                sc2 = qpool.tile([P, G], DT, tag="sq")
                nc.scalar.activation(
                    out=sc2[:cur],
                    in_=tnx[:cur],
                    func=mybir.ActivationFunctionType.Copy,
                    accum_out=st[:cur, 20 + t : 21 + t],
                )
                nc.vector.tensor_reduce(
                    out=st[:cur, 40 + t : 41 + t],
                    in_=trf[:cur, :],
                    op=mybir.AluOpType.min,
                    axis=X,
                )
                nc.vector.tensor_reduce(
                    out=st[:cur, 60 + t : 61 + t],
                    in_=trf[:cur, :],
                    op=mybir.AluOpType.max,
                    axis=X,
                )
                sq = qpool.tile([P, G], DT, tag="sq")
                nc.scalar.activation(
                    out=sq[:cur],
                    in_=trf[:cur],
                    func=mybir.ActivationFunctionType.Square,
                    accum_out=st[:cur, 80 + t : 81 + t],
                )
            nc.sync.dma_start(out=stats[:], in_=st[:])
    _split_waits(nc)
    return nc


def _build_pass2():
    import concourse.bass as bass
    import concourse.tile as tile
    from concourse import mybir

    DT = mybir.dt.float32
    nc = bass.Bass("TRN2", target_bir_lowering=False, debug=False, num_devices=NCORES)
    rf = nc.dram_tensor("rf", [RPC, G], DT, kind="ExternalInput")
    coef = nc.dram_tensor("coef", [RPC, 2], DT, kind="ExternalInput")
    out = nc.dram_tensor("out", [RPC, G], DT, kind="ExternalOutput")
    with tile.TileContext(nc) as tc:
        with tc.tile_pool(name="io", bufs=3) as pool:
            for t in range(NT):
                cur = min(P, RPC - t * P)
                trf = pool.tile([P, G], DT)
                nc.sync.dma_start(out=trf[:cur], in_=rf[t * P : t * P + cur])
                tcf = pool.tile([P, 2], DT)
                nc.sync.dma_start(out=tcf[:cur], in_=coef[t * P : t * P + cur])
                to = pool.tile([P, G], DT)
                nc.vector.tensor_scalar(
                    out=to[:cur],
                    in0=trf[:cur],
                    scalar1=tcf[:cur, 0:1],
                    scalar2=tcf[:cur, 1:2],
                    op0=mybir.AluOpType.mult,
                    op1=mybir.AluOpType.add,
                )
                # stores on the Activation HWDGE queue, loads on SP: the
                # two queues split the in/out streams
                nc.scalar.dma_start(out=out[t * P : t * P + cur], in_=to[:cur])
    _split_waits(nc)
    return nc


def _get_nc(which):
    if which not in _cache:
        _cache[which] = _build_pass1() if which == 1 else _build_pass2()
    return _cache[which]


def _random_draws():
    """The reference's MCMC random draws (jax threefry, key 42) are
    data-independent -> generate them once, bit-exactly, on the CPU."""
    if "draws" in _cache:
        return _cache["draws"]
    import jax

    cpu = jax.devices("cpu")[0]
    with jax.default_device(cpu):
        key = jax.random.key(42)
        k0, kloop = jax.random.split(key)
        states0 = np.asarray(jax.random.randint(k0, (N,), 0, 3))
        keys = jax.random.split(kloop, 20)
        cands = np.empty((20, N), np.int32)
        us = np.empty((20, N), np.float32)
        for t in range(20):
            k1, k2 = jax.random.split(keys[t])
            cands[t] = np.asarray(jax.random.randint(k1, (N,), 0, 3))
            us[t] = np.asarray(jax.random.uniform(k2, (N,)))
    _cache["draws"] = (states0, cands, us)
    return _cache["draws"]


def _mcmc_final_states(rfsum64, edge_index):
    """Replicates reference._mcmc_state_probs + scatter_mean + argmax.

    Output feeds the result only through a ~1e-11 relative term (the EPS in
    copy_sum + EPS), so f32 rounding differences here are immaterial; the
    random bits ARE exact."""
    m64 = rfsum64 / G
    mu = m64.mean()
    sd = m64.std()
    spot_mean = ((m64 - mu) / (sd + EPS)).astype(np.float32)
    m3 = m64[:3].astype(np.float32)

    d = spot_mean[None, :] - m3[:, None]
    L = np.exp(np.float32(-0.5) * d * d).sum(axis=1, dtype=np.float32)

    states0, cands, us = _random_draws()
    states = states0.copy()
    counts = np.zeros((N, 3), np.float32)
    idx = np.arange(N)
    for t in range(20):
        cand = cands[t]
        acc = (
            (_STATE_PRIOR[cand] / _STATE_PRIOR[states])
            * (L[cand] / L[states])
            * (_TRANS[states, cand] / _TRANS[cand, states])
        )
        states = np.where(us[t] < acc, cand, states)
        counts[idx, states] += 1.0

    row, col = np.asarray(edge_index[0]), np.asarray(edge_index[1])
    sums = np.zeros((N, 3), np.float32)
    np.add.at(sums, col, counts[row])
    cnt = np.bincount(col, minlength=N).astype(np.float32)
    smoothed = sums / np.maximum(cnt, 1.0)[:, None]
    return (np.argmax(smoothed, axis=1) + 1.0).astype(np.float32)


def kernel(norm_x, reconstructed_features, edge_index):
    from concourse.bass_utils import run_bass_kernel_spmd

    rf = np.ascontiguousarray(np.asarray(reconstructed_features, dtype=np.float32))
    nx = np.ascontiguousarray(np.asarray(norm_x, dtype=np.float32))
    core_ids = list(range(NCORES))

    in1 = [
        {"rf": rf[c * RPC : (c + 1) * RPC], "nx": nx[c * RPC : (c + 1) * RPC]}
        for c in range(NCORES)
    ]
    r1 = run_bass_kernel_spmd(_get_nc(1), in1, core_ids)

    def unpack(col0):
        full = np.concatenate(
            [
                r1.results[c]["stats"][:, col0 : col0 + 20].T.reshape(-1)[:RPC]
                for c in range(NCORES)
            ]
        )
        return full.astype(np.float64)

    rfsum = unpack(0)
    nxsum = unpack(20)
    rfmin = unpack(40)
    rfmax = unpack(60)
    sumsq = unpack(80)

    fs = _mcmc_final_states(rfsum, edge_index).astype(np.float64)

    # out[i,j] = rf[i,j]*scale_i normalized:  (x - mn)/(mx - mn + EPS) *
    # (1.2 mx - 0.8 mn) + 0.8 mn, all divided by its global mean
    scale = fs * nxsum / (fs * rfsum + EPS)
    mn = np.minimum(scale * rfmin, scale * rfmax).min()
    mx = np.maximum(scale * rfmin, scale * rfmax).max()
    a = (1.2 * mx - 0.8 * mn) / (mx - mn + EPS)
    b = 0.8 * mn - mn * a
    mean_nc0 = (scale * rfsum).sum() / (float(N) * float(G))
    m_all = a * mean_nc0 + b
    A = (scale * a / m_all).astype(np.float32)
    B = np.float32(b / m_all)

    coef = np.empty((N, 2), np.float32)
    coef[:, 0] = A
    coef[:, 1] = B

    in2 = [
        {"rf": rf[c * RPC : (c + 1) * RPC], "coef": coef[c * RPC : (c + 1) * RPC]}
        for c in range(NCORES)
    ]
    r2 = run_bass_kernel_spmd(_get_nc(2), in2, core_ids)
    out = np.concatenate([r2.results[c]["out"] for c in range(NCORES)], axis=0)

    reg_loss = np.float32(sumsq.sum() * 1e-4)
    return out, reg_loss


# revision 12
# speedup vs baseline: 68738.1167x; 59109.0869x over previous
"""Trainium2 Bass kernel for nn_CNEncoder_32658931319277.

Pipeline (see reference): per-spot stats over two [20000, 3000] f32
matrices, a tiny MCMC + graph smoothing on the [20000] spot dimension,
then an affine per-row rescale of `reconstructed_features` into the
output. The heavy O(N*G) work runs on 8 NeuronCores, data-parallel over
rows (2500 rows/core):

  pass 1 (device): per-row sum/min/max/sum-of-squares of rf + row sum
      of norm_x  -> tiny [128, 128] stats output per core.
  host glue (O(N+E), numpy/jax-cpu): MCMC with the exact jax threefry
      draws (data-independent), scatter-mean over edges, final_states,
      global min/max/mean algebra -> per-row A[i], global B with
      out[i,j] = rf[i,j] * A[i] + B.
  pass 2 (device): the affine map, one fused tensor_scalar per tile.
"""

import numpy as np

N = 20000
G = 3000
NCORES = 8
RPC = N // NCORES  # 2500 rows per core
P = 128
NT = (RPC + P - 1) // P  # 20 tiles, last tile 68 rows
CHUNKS = 2  # column chunks for the device row sums
EPS = 1e-8

_STATE_PRIOR = np.array([0.1, 0.8, 0.1], dtype=np.float32)
_TRANS = np.array(
    [[0.98, 0.01, 0.01], [0.01, 0.98, 0.01], [0.01, 0.01, 0.98]],
    dtype=np.float32,
)

_cache = {}


# --------------------------------------------------------------------------
# BIR post-pass: this walrus build accepts only ONE sync-wait command per
# instruction ("Too many sync wait commands" otherwise). Hoist excess waits
# into standalone single-wait EventSemaphore instructions on the same engine.
# --------------------------------------------------------------------------
def _split_waits(nc, max_waits=1):
    import concourse.mybir as mybir

    n = 0
    for f in nc.m.functions:
        for bb in f.blocks:
            new_insts = []
            changed = False
            for inst in bb.instructions:
                si = getattr(inst, "sync_info", None)
                waits = list(si.on_wait) if si is not None and si.on_wait else []
                if len(waits) > max_waits:
                    hoist, keep = waits[:-max_waits], waits[-max_waits:]
                    for w in hoist:
                        n += 1
                        new_insts.append(
                            mybir.InstEventSemaphore(
                                name=f"wsplit-{n}-{inst.name}",
                                engine=inst.engine,
                                ins=[],
                                outs=[],
                                sync_info=mybir.SyncInfo(on_wait=[w], on_update=[]),
                            )
                        )
                    si.on_wait = keep
                    changed = True
                new_insts.append(inst)
            if changed:
                bb.instructions[:] = new_insts
    return n


def _build_pass1():
    import concourse.bass as bass
    import concourse.tile as tile
    from concourse import mybir

    DT = mybir.dt.float32
    X = mybir.AxisListType.X
    CH = CHUNKS  # row-sum column chunks: shorter f32 accumulation runs
    CS = G // CH
    nc = bass.Bass("TRN2", target_bir_lowering=False, debug=False, num_devices=NCORES)
    rf = nc.dram_tensor("rf", [RPC, G], DT, kind="ExternalInput")
    nx = nc.dram_tensor("nx", [RPC, G], DT, kind="ExternalInput")
    stats = nc.dram_tensor("stats", [P, 256], DT, kind="ExternalOutput")
    # stats columns (row p of tile t <-> shard row t*128+p):
    #   CH*t+k      k-th column-chunk partial of rfsum   (k in [0,CH))
    #   80+CH*t+k   k-th column-chunk partial of nxsum
    #   160+t       rfmin    180+t  rfmax    200+t  rf sum of squares
    with tile.TileContext(nc) as tc:
        with (
            tc.tile_pool(name="io", bufs=3) as pool,
            tc.tile_pool(name="scratch", bufs=2) as qpool,
            tc.tile_pool(name="ttrp", bufs=2) as tpool,
            tc.tile_pool(name="stat", bufs=1) as spool,
        ):
            st = spool.tile([P, 256], DT)
            nc.vector.memset(st[:], 0.0)
            for t in range(NT):
                cur = min(P, RPC - t * P)
                trf = pool.tile([P, G], DT)
                nc.sync.dma_start(out=trf[:cur], in_=rf[t * P : t * P + cur])
                tnx = pool.tile([P, G], DT)
                nc.sync.dma_start(out=tnx[:cur], in_=nx[t * P : t * P + cur])
                # chunked row sums on ACT (Copy with fp32 accumulate);
                # min/max + sum-of-squares on DVE. DMA-bound overall.
                for k in range(CH):
                    sc = qpool.tile([P, G], DT, tag="sq")
                    nc.scalar.activation(
                        out=sc[:cur, :CS],
                        in_=trf[:cur, k * CS : (k + 1) * CS],
                        func=mybir.ActivationFunctionType.Copy,
                        accum_out=st[:cur, CH * t + k : CH * t + k + 1],
                    )
                    sc2 = qpool.tile([P, G], DT, tag="sq")
                    nc.scalar.activation(
                        out=sc2[:cur, :CS],
                        in_=tnx[:cur, k * CS : (k + 1) * CS],
                        func=mybir.ActivationFunctionType.Copy,
                        accum_out=st[:cur, 80 + CH * t + k : 80 + CH * t + k + 1],
                    )
                nc.vector.tensor_reduce(
                    out=st[:cur, 160 + t : 161 + t],
                    in_=trf[:cur, :],
                    op=mybir.AluOpType.min,
                    axis=X,
                )
                nc.vector.tensor_reduce(
                    out=st[:cur, 180 + t : 181 + t],
                    in_=trf[:cur, :],
                    op=mybir.AluOpType.max,
                    axis=X,
                )
                sqt = tpool.tile([P, G], DT, tag="ttr")
                nc.scalar.activation(
                    out=sqt[:cur],
                    in_=trf[:cur],
                    func=mybir.ActivationFunctionType.Square,
                    accum_out=st[:cur, 200 + t : 201 + t],
                )
            nc.sync.dma_start(out=stats[:], in_=st[:])
    _split_waits(nc)
    return nc


def _build_pass2():
    import concourse.bass as bass
    import concourse.tile as tile
    from concourse import mybir

    DT = mybir.dt.float32
    nc = bass.Bass("TRN2", target_bir_lowering=False, debug=False, num_devices=NCORES)
    rf = nc.dram_tensor("rf", [RPC, G], DT, kind="ExternalInput")
    coef = nc.dram_tensor("coef", [RPC, 2], DT, kind="ExternalInput")
    out = nc.dram_tensor("out", [RPC, G], DT, kind="ExternalOutput")
    with tile.TileContext(nc) as tc:
        with tc.tile_pool(name="io", bufs=3) as pool:
            for t in range(NT):
                cur = min(P, RPC - t * P)
                trf = pool.tile([P, G], DT)
                nc.sync.dma_start(out=trf[:cur], in_=rf[t * P : t * P + cur])
                tcf = pool.tile([P, 2], DT)
                nc.sync.dma_start(out=tcf[:cur], in_=coef[t * P : t * P + cur])
                to = pool.tile([P, G], DT)
                nc.vector.tensor_scalar(
                    out=to[:cur],
                    in0=trf[:cur],
                    scalar1=tcf[:cur, 0:1],
                    scalar2=tcf[:cur, 1:2],
                    op0=mybir.AluOpType.mult,
                    op1=mybir.AluOpType.add,
                )
                # stores on the Activation HWDGE queue, loads on SP: the
                # two queues split the in/out streams
                nc.scalar.dma_start(out=out[t * P : t * P + cur], in_=to[:cur])
    _split_waits(nc)
    return nc


def _get_nc(which):
    if which not in _cache:
        _cache[which] = _build_pass1() if which == 1 else _build_pass2()
    return _cache[which]


def _random_draws():
    """The reference's MCMC random draws (jax threefry, key 42) are
    data-independent -> generate them once, bit-exactly, on the CPU."""
    if "draws" in _cache:
        return _cache["draws"]
    import jax

    cpu = jax.devices("cpu")[0]
    with jax.default_device(cpu):
        key = jax.random.key(42)
        k0, kloop = jax.random.split(key)
        states0 = np.asarray(jax.random.randint(k0, (N,), 0, 3))
        keys = jax.random.split(kloop, 20)
        cands = np.empty((20, N), np.int32)
        us = np.empty((20, N), np.float32)
        for t in range(20):
            k1, k2 = jax.random.split(keys[t])
            cands[t] = np.asarray(jax.random.randint(k1, (N,), 0, 3))
            us[t] = np.asarray(jax.random.uniform(k2, (N,)))
    _cache["draws"] = (states0, cands, us)
    return _cache["draws"]


def _mcmc_final_states(rfsum64, edge_index):
    """Replicates reference._mcmc_state_probs + scatter_mean + argmax.

    Output feeds the result only through a ~1e-11 relative term (the EPS in
    copy_sum + EPS), so f32 rounding differences here are immaterial; the
    random bits ARE exact."""
    m64 = rfsum64 / G
    mu = m64.mean()
    sd = m64.std()
    spot_mean = ((m64 - mu) / (sd + EPS)).astype(np.float32)
    m3 = m64[:3].astype(np.float32)

    d = spot_mean[None, :] - m3[:, None]
    L = np.exp(np.float32(-0.5) * d * d).sum(axis=1, dtype=np.float32)

    states0, cands, us = _random_draws()
    states = states0.copy()
    counts = np.zeros((N, 3), np.float32)
    idx = np.arange(N)
    for t in range(20):
        cand = cands[t]
        acc = (
            (_STATE_PRIOR[cand] / _STATE_PRIOR[states])
            * (L[cand] / L[states])
            * (_TRANS[states, cand] / _TRANS[cand, states])
        )
        states = np.where(us[t] < acc, cand, states)
        counts[idx, states] += 1.0

    row, col = np.asarray(edge_index[0]), np.asarray(edge_index[1])
    sums = np.zeros((N, 3), np.float32)
    np.add.at(sums, col, counts[row])
    cnt = np.bincount(col, minlength=N).astype(np.float32)
    smoothed = sums / np.maximum(cnt, 1.0)[:, None]
    return (np.argmax(smoothed, axis=1) + 1.0).astype(np.float32)


def kernel(norm_x, reconstructed_features, edge_index):
    from concourse.bass_utils import run_bass_kernel_spmd

    rf = np.ascontiguousarray(np.asarray(reconstructed_features, dtype=np.float32))
    nx = np.ascontiguousarray(np.asarray(norm_x, dtype=np.float32))
    core_ids = list(range(NCORES))

    in1 = [
        {"rf": rf[c * RPC : (c + 1) * RPC], "nx": nx[c * RPC : (c + 1) * RPC]}
        for c in range(NCORES)
    ]
    r1 = run_bass_kernel_spmd(_get_nc(1), in1, core_ids)

    def unpack(col0):
        full = np.concatenate(
            [
                r1.results[c]["stats"][:, col0 : col0 + 20].T.reshape(-1)[:RPC]
                for c in range(NCORES)
            ]
        )
        return full.astype(np.float64)

    def unpack_chunked(col0):
        # col0 + CHUNKS*t + k holds chunk k of tile t; combine in fp64
        full = np.concatenate(
            [
                r1.results[c]["stats"][:, col0 : col0 + CHUNKS * NT]
                .astype(np.float64)
                .reshape(P, NT, CHUNKS)
                .sum(axis=2)
                .T.reshape(-1)[:RPC]
                for c in range(NCORES)
            ]
        )
        return full

    rfsum = unpack_chunked(0)
    nxsum = unpack_chunked(80)
    rfmin = unpack(160)
    rfmax = unpack(180)
    sumsq = unpack(200)

    fs = _mcmc_final_states(rfsum, edge_index).astype(np.float64)

    # out[i,j] = rf[i,j]*scale_i normalized:  (x - mn)/(mx - mn + EPS) *
    # (1.2 mx - 0.8 mn) + 0.8 mn, all divided by its global mean
    scale = fs * nxsum / (fs * rfsum + EPS)
    mn = np.minimum(scale * rfmin, scale * rfmax).min()
    mx = np.maximum(scale * rfmin, scale * rfmax).max()
    a = (1.2 * mx - 0.8 * mn) / (mx - mn + EPS)
    b = 0.8 * mn - mn * a
    mean_nc0 = (scale * rfsum).sum() / (float(N) * float(G))
    m_all = a * mean_nc0 + b
    A = (scale * a / m_all).astype(np.float32)
    B = np.float32(b / m_all)

    coef = np.empty((N, 2), np.float32)
    coef[:, 0] = A
    coef[:, 1] = B

    in2 = [
        {"rf": rf[c * RPC : (c + 1) * RPC], "coef": coef[c * RPC : (c + 1) * RPC]}
        for c in range(NCORES)
    ]
    r2 = run_bass_kernel_spmd(_get_nc(2), in2, core_ids)
    out = np.concatenate([r2.results[c]["out"] for c in range(NCORES)], axis=0)

    reg_loss = np.float32(sumsq.sum() * 1e-4)
    return out, reg_loss


# revision 19
# speedup vs baseline: 71507.6243x; 1.0403x over previous
"""Trainium2 Bass kernel for nn_CNEncoder_32658931319277.

Pipeline (see reference): per-spot stats over two [20000, 3000] f32
matrices, a tiny MCMC + graph smoothing on the [20000] spot dimension,
then an affine per-row rescale of `reconstructed_features` into the
output. The heavy O(N*G) work runs on 8 NeuronCores, data-parallel over
rows (2500 rows/core):

  pass 1 (device): per-row (chunked) sum/min/max of rf + row sum of
      norm_x  -> tiny [128, 256] stats output per core.
  host glue (O(N+E), numpy/jax-cpu): MCMC with the exact jax threefry
      draws (data-independent), scatter-mean over edges, final_states,
      global min/max/mean algebra -> per-row A[i], global B with
      out[i,j] = rf[i,j] * A[i] + B.
  pass 2 (device): the affine map, one fused tensor_scalar per tile,
      plus rf sum-of-squares (reg_loss) on the otherwise-idle ACT engine.
"""

import numpy as np

N = 20000
G = 3000
NCORES = 8
RPC = N // NCORES  # 2500 rows per core
P = 128
NT = (RPC + P - 1) // P  # 20 tiles, last tile 68 rows
CHUNKS = 4  # column chunks for the device row sums
EPS = 1e-8

_STATE_PRIOR = np.array([0.1, 0.8, 0.1], dtype=np.float32)
_TRANS = np.array(
    [[0.98, 0.01, 0.01], [0.01, 0.98, 0.01], [0.01, 0.01, 0.98]],
    dtype=np.float32,
)

_cache = {}


# --------------------------------------------------------------------------
# BIR post-pass: this walrus build accepts only ONE sync-wait command per
# instruction ("Too many sync wait commands" otherwise). Hoist excess waits
# into standalone single-wait EventSemaphore instructions on the same engine.
# --------------------------------------------------------------------------
def _split_waits(nc, max_waits=1):
    import concourse.mybir as mybir

    n = 0
    for f in nc.m.functions:
        for bb in f.blocks:
            new_insts = []
            changed = False
            for inst in bb.instructions:
                si = getattr(inst, "sync_info", None)
                waits = list(si.on_wait) if si is not None and si.on_wait else []
                if len(waits) > max_waits:
                    hoist, keep = waits[:-max_waits], waits[-max_waits:]
                    for w in hoist:
                        n += 1
                        new_insts.append(
                            mybir.InstEventSemaphore(
                                name=f"wsplit-{n}-{inst.name}",
                                engine=inst.engine,
                                ins=[],
                                outs=[],
                                sync_info=mybir.SyncInfo(on_wait=[w], on_update=[]),
                            )
                        )
                    si.on_wait = keep
                    changed = True
                new_insts.append(inst)
            if changed:
                bb.instructions[:] = new_insts
    return n


def _build_pass1():
    import concourse.bass as bass
    import concourse.tile as tile
    from concourse import mybir

    DT = mybir.dt.float32
    X = mybir.AxisListType.X
    CH = CHUNKS  # row-sum column chunks: shorter f32 accumulation runs
    CS = G // CH
    nc = bass.Bass("TRN2", target_bir_lowering=False, debug=False, num_devices=NCORES)
    rf = nc.dram_tensor("rf", [RPC, G], DT, kind="ExternalInput")
    nx = nc.dram_tensor("nx", [RPC, G], DT, kind="ExternalInput")
    stats = nc.dram_tensor("stats", [P, 256], DT, kind="ExternalOutput")
    # stats columns (row p of tile t <-> shard row t*128+p):
    #   CH*t+k      k-th column-chunk partial of rfsum   (k in [0,CH))
    #   80+CH*t+k   k-th column-chunk partial of nxsum
    #   160+t       rfmin    180+t  rfmax
    # (sum of squares for reg_loss is computed in pass 2, where ACT is idle)
    with tile.TileContext(nc) as tc:
        with (
            tc.tile_pool(name="io", bufs=3) as pool,
            tc.tile_pool(name="scratch", bufs=2) as qpool,
            tc.tile_pool(name="stat", bufs=1) as spool,
        ):
            st = spool.tile([P, 256], DT)
            nc.vector.memset(st[:], 0.0)
            for t in range(NT):
                cur = min(P, RPC - t * P)
                trf = pool.tile([P, G], DT)
                nc.sync.dma_start(out=trf[:cur], in_=rf[t * P : t * P + cur])
                tnx = pool.tile([P, G], DT)
                nc.sync.dma_start(out=tnx[:cur], in_=nx[t * P : t * P + cur])
                # chunked row sums on ACT (Copy with fp32 accumulate);
                # min/max + sum-of-squares on DVE. DMA-bound overall.
                for k in range(CH):
                    sc = qpool.tile([P, G], DT, tag="sq")
                    nc.scalar.activation(
                        out=sc[:cur, :CS],
                        in_=trf[:cur, k * CS : (k + 1) * CS],
                        func=mybir.ActivationFunctionType.Copy,
                        accum_out=st[:cur, CH * t + k : CH * t + k + 1],
                    )
                    sc2 = qpool.tile([P, G], DT, tag="sq")
                    nc.scalar.activation(
                        out=sc2[:cur, :CS],
                        in_=tnx[:cur, k * CS : (k + 1) * CS],
                        func=mybir.ActivationFunctionType.Copy,
                        accum_out=st[:cur, 80 + CH * t + k : 80 + CH * t + k + 1],
                    )
                nc.vector.tensor_reduce(
                    out=st[:cur, 160 + t : 161 + t],
                    in_=trf[:cur, :],
                    op=mybir.AluOpType.min,
                    axis=X,
                )
                nc.vector.tensor_reduce(
                    out=st[:cur, 180 + t : 181 + t],
                    in_=trf[:cur, :],
                    op=mybir.AluOpType.max,
                    axis=X,
                )
            nc.sync.dma_start(out=stats[:], in_=st[:])
    _split_waits(nc)
    return nc


def _build_pass2():
    import concourse.bass as bass
    import concourse.tile as tile
    from concourse import mybir

    DT = mybir.dt.float32
    nc = bass.Bass("TRN2", target_bir_lowering=False, debug=False, num_devices=NCORES)
    rf = nc.dram_tensor("rf", [RPC, G], DT, kind="ExternalInput")
    coef = nc.dram_tensor("coef", [RPC, 2], DT, kind="ExternalInput")
    out = nc.dram_tensor("out", [RPC, G], DT, kind="ExternalOutput")
    stats2 = nc.dram_tensor("stats2", [P, 32], DT, kind="ExternalOutput")
    with tile.TileContext(nc) as tc:
        with (
            tc.tile_pool(name="io", bufs=3) as pool,
            tc.tile_pool(name="sqp", bufs=2) as qpool,
            tc.tile_pool(name="stat", bufs=1) as spool,
        ):
            st2 = spool.tile([P, 32], DT)
            nc.vector.memset(st2[:], 0.0)
            for t in range(NT):
                cur = min(P, RPC - t * P)
                trf = pool.tile([P, G], DT)
                nc.sync.dma_start(out=trf[:cur], in_=rf[t * P : t * P + cur])
                tcf = pool.tile([P, 2], DT)
                nc.sync.dma_start(out=tcf[:cur], in_=coef[t * P : t * P + cur])
                to = pool.tile([P, G], DT)
                nc.vector.tensor_scalar(
                    out=to[:cur],
                    in0=trf[:cur],
                    scalar1=tcf[:cur, 0:1],
                    scalar2=tcf[:cur, 1:2],
                    op0=mybir.AluOpType.mult,
                    op1=mybir.AluOpType.add,
                )
                # rf sum-of-squares for reg_loss rides along on the
                # otherwise-idle ACT engine (col t of stats2)
                sq = qpool.tile([P, G], DT, tag="sq")
                nc.scalar.activation(
                    out=sq[:cur],
                    in_=trf[:cur],
                    func=mybir.ActivationFunctionType.Square,
                    accum_out=st2[:cur, t : t + 1],
                )
                # stores on the Activation HWDGE queue, loads on SP: the
                # two queues split the in/out streams
                nc.scalar.dma_start(out=out[t * P : t * P + cur], in_=to[:cur])
            nc.sync.dma_start(out=stats2[:], in_=st2[:])
    _split_waits(nc)
    return nc


def _get_nc(which):
    if which not in _cache:
        _cache[which] = _build_pass1() if which == 1 else _build_pass2()
    return _cache[which]


def _random_draws():
    """The reference's MCMC random draws (jax threefry, key 42) are
    data-independent -> generate them once, bit-exactly, on the CPU."""
    if "draws" in _cache:
        return _cache["draws"]
    import jax

    cpu = jax.devices("cpu")[0]
    with jax.default_device(cpu):
        key = jax.random.key(42)
        k0, kloop = jax.random.split(key)
        states0 = np.asarray(jax.random.randint(k0, (N,), 0, 3))
        keys = jax.random.split(kloop, 20)
        cands = np.empty((20, N), np.int32)
        us = np.empty((20, N), np.float32)
        for t in range(20):
            k1, k2 = jax.random.split(keys[t])
            cands[t] = np.asarray(jax.random.randint(k1, (N,), 0, 3))
            us[t] = np.asarray(jax.random.uniform(k2, (N,)))
    _cache["draws"] = (states0, cands, us)
    return _cache["draws"]


def _mcmc_final_states(rfsum64, edge_index):
    """Replicates reference._mcmc_state_probs + scatter_mean + argmax.

    Output feeds the result only through a ~1e-11 relative term (the EPS in
    copy_sum + EPS), so f32 rounding differences here are immaterial; the
    random bits ARE exact."""
    m64 = rfsum64 / G
    mu = m64.mean()
    sd = m64.std()
    spot_mean = ((m64 - mu) / (sd + EPS)).astype(np.float32)
    m3 = m64[:3].astype(np.float32)

    d = spot_mean[None, :] - m3[:, None]
    L = np.exp(np.float32(-0.5) * d * d).sum(axis=1, dtype=np.float32)

    states0, cands, us = _random_draws()
    states = states0.copy()
    counts = np.zeros((N, 3), np.float32)
    idx = np.arange(N)
    for t in range(20):
        cand = cands[t]
        acc = (
            (_STATE_PRIOR[cand] / _STATE_PRIOR[states])
            * (L[cand] / L[states])
            * (_TRANS[states, cand] / _TRANS[cand, states])
        )
        states = np.where(us[t] < acc, cand, states)
        counts[idx, states] += 1.0

    row, col = np.asarray(edge_index[0]), np.asarray(edge_index[1])
    sums = np.zeros((N, 3), np.float32)
    np.add.at(sums, col, counts[row])
    cnt = np.bincount(col, minlength=N).astype(np.float32)
    smoothed = sums / np.maximum(cnt, 1.0)[:, None]
    return (np.argmax(smoothed, axis=1) + 1.0).astype(np.float32)


def kernel(norm_x, reconstructed_features, edge_index):
    from concourse.bass_utils import run_bass_kernel_spmd

    rf = np.ascontiguousarray(np.asarray(reconstructed_features, dtype=np.float32))
    nx = np.ascontiguousarray(np.asarray(norm_x, dtype=np.float32))
    core_ids = list(range(NCORES))

    in1 = [
        {"rf": rf[c * RPC : (c + 1) * RPC], "nx": nx[c * RPC : (c + 1) * RPC]}
        for c in range(NCORES)
    ]
    r1 = run_bass_kernel_spmd(_get_nc(1), in1, core_ids)

    def unpack(col0):
        full = np.concatenate(
            [
                r1.results[c]["stats"][:, col0 : col0 + 20].T.reshape(-1)[:RPC]
                for c in range(NCORES)
            ]
        )
        return full.astype(np.float64)

    def unpack_chunked(col0):
        # col0 + CHUNKS*t + k holds chunk k of tile t; combine in fp64
        full = np.concatenate(
            [
                r1.results[c]["stats"][:, col0 : col0 + CHUNKS * NT]
                .astype(np.float64)
                .reshape(P, NT, CHUNKS)
                .sum(axis=2)
                .T.reshape(-1)[:RPC]
                for c in range(NCORES)
            ]
        )
        return full

    rfsum = unpack_chunked(0)
    nxsum = unpack_chunked(80)
    rfmin = unpack(160)
    rfmax = unpack(180)

    fs = _mcmc_final_states(rfsum, edge_index).astype(np.float64)

    # out[i,j] = rf[i,j]*scale_i normalized:  (x - mn)/(mx - mn + EPS) *
    # (1.2 mx - 0.8 mn) + 0.8 mn, all divided by its global mean
    scale = fs * nxsum / (fs * rfsum + EPS)
    mn = np.minimum(scale * rfmin, scale * rfmax).min()
    mx = np.maximum(scale * rfmin, scale * rfmax).max()
    a = (1.2 * mx - 0.8 * mn) / (mx - mn + EPS)
    b = 0.8 * mn - mn * a
    mean_nc0 = (scale * rfsum).sum() / (float(N) * float(G))
    m_all = a * mean_nc0 + b
    A = (scale * a / m_all).astype(np.float32)
    B = np.float32(b / m_all)

    coef = np.empty((N, 2), np.float32)
    coef[:, 0] = A
    coef[:, 1] = B

    in2 = [
        {"rf": rf[c * RPC : (c + 1) * RPC], "coef": coef[c * RPC : (c + 1) * RPC]}
        for c in range(NCORES)
    ]
    r2 = run_bass_kernel_spmd(_get_nc(2), in2, core_ids)
    out = np.concatenate([r2.results[c]["out"] for c in range(NCORES)], axis=0)

    sumsq = np.concatenate(
        [
            r2.results[c]["stats2"][:, 0:NT].T.reshape(-1)[:RPC]
            for c in range(NCORES)
        ]
    ).astype(np.float64)
    reg_loss = np.float32(sumsq.sum() * 1e-4)
    return out, reg_loss
